# revision 20
# baseline (speedup 1.0000x reference)
"""Causal single-head attention (B=4, S=4096, D=1024, d_key=64) on 8 trn2 cores.

Sharding: 8 cores = 4 batches x 2 KEY-halves. Core (b, h) holds ALL 4096 query
rows of batch b but only the key/value 128-row blocks {j : j % 2 == h} (2048
keys, interleaved for causal balance). Each core computes the partial softmax
accumulator (unnormalized numerator + denominator row) of every query row over
its own key half; the HOST adds the two halves of each pair and normalizes.
No cross-core communication, and K/V raw loads + projections are not
replicated (the baseline replicated both).

DMA diet: queries and keys stream in as fp8 e3m4 (4 mantissa bits); the
projection weights Wq/Wk are pre-scaled by 64 on the host so their range
suits e3m4, and the 1/64^2 is folded into the softmax exp scale. Values and
Wv stay bf16 (V-path quantization hits the output linearly; the score path
is softened by softmax). Measured end-to-end rel-err ~1e-2 vs fp64.

Device kernel (identical SPMD program; per-core differences are input data):
  1. Project qT [64, 4096] and kT [64, 2048] (weights stationary, e3m4 data,
     fp32 PSUM, stored bf16) and v-natural [128, 65]-blocks (data stationary
     -> natural PV layout; col 64 is a ones column for the denominator).
  2. CHUNK-major attention: for q chunk c (256 rows), own-key blocks m=0..c
     (causal; the packed block m maps to global block 2m+h, so the count and
     the boundary structure are core-independent): score matmuls in groups of
     up to 4 blocks -> one ACT exp per group (scale folds the 64x64 weight
     scaling) -> boundary mask (a single constant [128,256] tile, only block
     m==c needs it) -> PV matmuls ACCUMULATE the whole chunk in one PSUM tile
     [65, 256] (no SBUF accumulator, no DVE adds).
  3. The chunk accumulator DMAs straight from PSUM to DRAM via the POOL DGE
     (last two chunks via HWDGE); host combines + normalizes + transposes.
"""

import numpy as np

import concourse.mybir as mybir
import concourse.tile as tile
from concourse import bacc
from concourse.bass_utils import run_bass_kernel_spmd

B, S, D, DK = 4, 4096, 1024, 64
NCORES = 8
CH = 256  # query rows per chunk
NCH = 16  # chunks per core (all rows)
KB = 2048  # own keys per core
JB = 128  # key block
NKB = KB // JB  # 16 own key blocks
DC = D // 128  # 8 contraction chunks
F32 = mybir.dt.float32
BF16 = mybir.dt.bfloat16
E3 = mybir.dt.float8e3
WSCALE = 64.0  # host pre-scales Wq/Wk by this; folded into exp scale
SCALE = 0.125 / (WSCALE * WSCALE)

_prog_cache = {}


def _build(variant):
    causal = variant == "causal"
    nkq = [c + 1 if causal else NKB for c in range(NCH)]  # own blocks/chunk

    nc = bacc.Bacc("TRN2", target_bir_lowering=False, debug=False,
                   num_devices=NCORES)

    qt_d = nc.declare_dram_parameter("qt", [D, S], E3, isOutput=False)
    kt_d = nc.declare_dram_parameter("kt", [D, KB], E3, isOutput=False)
    vt_d = nc.declare_dram_parameter("vt", [D, KB], BF16, isOutput=False)
    wq_d = nc.declare_dram_parameter("wq", [D, DK], E3, isOutput=False)
    wk_d = nc.declare_dram_parameter("wk", [D, DK], E3, isOutput=False)
    wv_d = nc.declare_dram_parameter("wv", [D, DK], BF16, isOutput=False)
    if causal:
        mask_d = nc.declare_dram_parameter("maskb", [JB, 2, JB], BF16,
                                           isOutput=False)
    # raw transposed partial accumulators (+denominator row); host combines
    out_d = nc.declare_dram_parameter("out", [NCH, DK + 1, CH], F32,
                                      isOutput=True)

    NSQ = S // 512  # 8 column groups of 512 for q
    NSK = KB // 512  # 4 groups for k/v

    qt3 = qt_d.rearrange("(o p) s -> p o s", p=128)
    kt3 = kt_d.rearrange("(o p) s -> p o s", p=128)
    vt3 = vt_d.rearrange("(o p) s -> p o s", p=128)

    with tile.TileContext(nc) as tc:
        with (
            tc.tile_pool(name="const", bufs=1) as const,
            tc.tile_pool(name="res", bufs=1) as res,
            tc.tile_pool(name="stage", bufs=10) as stage,
            tc.tile_pool(name="pwork", bufs=4) as pwork,
            tc.tile_pool(name="ps_mm", bufs=2, space="PSUM") as ps_mm,
            tc.tile_pool(name="ps_s", bufs=2, space="PSUM") as ps_s,
            tc.tile_pool(name="ps_o", bufs=2, space="PSUM") as ps_o,
        ):
            def stage_load(src3, sc, dt, splits=2):
                """Split-group DMAs so the first matmuls start early."""
                w = DC // splits
                sts = []
                for hh in range(splits):
                    st = stage.tile([128, w, 512], dt, tag="stage",
                                    name=f"st{hh}")
                    nc.sync.dma_start(
                        st[:],
                        src3[:, w * hh:w * (hh + 1), sc * 512:(sc + 1) * 512])
                    sts.append(st)
                return sts

            bg = []  # background projection thunks, woven between attn units

            def project_qk(kind, src3, w_sb, nat, base, sc, xbars,
                           sts=None, defer=False):
                """One 512-column group projected DATA-stationary (4 cyc/row)
                into the natural [row, 64] layout, then XBAR DMA-transposed
                into split-partition qT/kT pair tiles (on the ACT HWDGE)."""
                if sts is None:
                    sts = stage_load(src3, sc, E3)
                w = DC // len(sts)
                box = {}

                def mm(sb):
                    if sb == 0:
                        box["ps"] = ps_mm.tile([128, 4, DK], F32, tag="mm",
                                               name="ps_qk")
                    for dc in range(DC):
                        nc.tensor.matmul(
                            box["ps"][:, sb, :],
                            sts[dc // w][:, dc % w,
                                         sb * 128:(sb + 1) * 128],
                            w_sb[:, dc, :],
                            start=(dc == 0), stop=(dc == DC - 1))
                    nc.vector.tensor_copy(
                        nat[:, base + 4 * sc + sb, :], box["ps"][:, sb, :])

                def xb():
                    for dst, ni in xbars:
                        npair = dst.shape[1]
                        nc.scalar.dma_start_transpose(
                            dst, nat[:, ni:ni + 2 * npair, :])


                for sb in range(4):
                    if defer:
                        bg.append(((kind, sc), lambda sb=sb: mm(sb)))
                    else:
                        mm(sb)
                if defer:
                    bg.append(((kind, sc), xb))
                else:
                    xb()

            def project_v(sc, sts=None, defer=False):
                """V projected directly to natural [s, c] blocks: lhsT is the
                staged data chunk, rhs the weights -> out [128 s, 64 c], which
                is exactly the PV lhsT layout."""
                if sts is None:
                    sts = stage_load(vt3, sc, BF16)
                w = DC // len(sts)
                box = {}

                def mm(sb):
                    if sb == 0:
                        box["ps"] = ps_mm.tile([128, 4, DK], F32, tag="mm",
                                               name="ps_v")
                    for dc in range(DC):
                        nc.tensor.matmul(
                            box["ps"][:, sb, :],
                            sts[dc // w][:, dc % w,
                                         sb * 128:(sb + 1) * 128],
                            wv_sb[:, dc, :],
                            start=(dc == 0), stop=(dc == DC - 1))
                    nc.vector.tensor_copy(vgs[sc][:, sb, 0:DK],
                                          box["ps"][:, sb, :])
                    if sb == 3:
                        nc.vector.memset(vgs[sc][:, :, DK:DK + 1], 1.0)

                for sb in range(4):
                    if defer:
                        bg.append((("v", sc), lambda sb=sb: mm(sb)))
                    else:
                        mm(sb)

            # PE warm-up in the initial DMA shadow
            warm = const.tile([128, 512], BF16, tag="warm")
            nc.vector.memset(warm[:], 0.0)
            for _ in range(8):
                wps = ps_mm.tile([DK, 512], F32, tag="mm", name="wps")
                nc.tensor.matmul(wps[:], warm[:, 0:DK], warm[:],
                                 start=True, stop=True)
            wq_sb = const.tile([128, DC, DK], E3, tag="wq")
            wk_sb = const.tile([128, DC, DK], E3, tag="wk")
            wv_sb = const.tile([128, DC, DK], BF16, tag="wv")
            head_q0 = stage_load(qt3, 0, E3)
            nc.sync.dma_start(wq_sb[:], wq_d.rearrange("(o p) c -> p o c", p=128))
            nc.sync.dma_start(wk_sb[:], wk_d.rearrange("(o p) c -> p o c", p=128))
            nc.sync.dma_start(wv_sb[:], wv_d.rearrange("(o p) c -> p o c", p=128))
            head_k0 = stage_load(kt3, 0, E3)
            head_v0 = stage_load(vt3, 0, BF16)
            if causal:
                msk_sb = const.tile([JB, 2, JB], BF16, tag="msk")
                nc.sync.dma_start(msk_sb[:], mask_d[:])

            # natural projected q/k [row%128, block, dim] bf16; knat has a
            # pad block on each end so every XBAR slab (j-1, j) exists
            qnat = res.tile([128, 2 * NCH, DK], BF16, tag="qnat")
            knat = res.tile([128, NKB + 2, DK], BF16, tag="knat")
            nc.vector.memset(knat[:, 0, :], 0.0)
            nc.vector.memset(knat[:, NKB + 1, :], 0.0)
            # XBAR-transposed pair tiles, one tile per XBAR call so the
            # tile-granular dependency tracker sees no false deps:
            # qxg[g][:, j, :] = pair for chunk 2g+j; slab p (= kT pair
            # (p-1, p)) lives in kxg[p//4, p%2][:, (p%4)//2, :]
            qxg = [res.tile([128, 2, JB], BF16, tag=f"qx{g}", name=f"qx{g}")
                   for g in range(NSQ)]
            kxg = [[res.tile([128, 2, JB], BF16, tag=f"kx{g}e", name=f"kx{g}e"),
                    res.tile([128, 2, JB], BF16, tag=f"kx{g}o", name=f"kx{g}o")]
                   for g in range(NSK)]
            kxf = res.tile([128, 1, JB], BF16, tag="kxf")

            def kslab(p):
                if p == NKB:
                    return kxf[:, 0, :]
                return kxg[p // 4][p % 2][:, (p % 4) // 2, :]
            # v natural (+ones col): per 512-group, 4 blocks of [128, 65]
            vgs = [res.tile([128, 4, DK + 1], BF16, tag=f"vg{sc}",
                            name=f"vg{sc}")
                   for sc in range(NSK)]

            def emit_unit(c, m0, nb, o_ps, first):
                nb_tot = nkq[c]
                # half-major layout: the two q-halves use different PE row
                # bases (0 / 64), and mixed row-base matmuls must not share a
                # PSUM bank -> half h goes to its own bank of the tile
                s_ps = ps_s.tile([128, 2, 4, JB], F32, tag="s", name="s_ps")
                for i in range(nb):
                    m = m0 + i
                    # kT_m lives at partitions 0:64 of pair tile m+1 and at
                    # 64:128 of pair tile m; q halves sit at matching bases
                    qx = qxg[c // 2][:, c % 2, :]
                    nc.tensor.matmul(
                        s_ps[:, 0, i, :], kslab(m + 1)[0:DK, :],
                        qx[0:DK, :], start=True, stop=True)
                    nc.tensor.matmul(
                        s_ps[:, 1, i, :], kslab(m)[DK:2 * DK, :],
                        qx[DK:2 * DK, :], start=True, stop=True)
                if bg:
                    bg.pop(0)[1]()
                p_sb = pwork.tile([128, 2, nb, JB], BF16, tag="p",
                                  name=f"p{nb}")
                nc.scalar.activation(p_sb[:], s_ps[:, :, 0:nb, :],
                                     mybir.ActivationFunctionType.Exp,
                                     scale=SCALE)
                if causal and m0 + nb == nb_tot:
                    # boundary block is always the chunk's last block
                    nc.vector.tensor_mul(p_sb[:, :, nb - 1, :],
                                         p_sb[:, :, nb - 1, :], msk_sb[:])
                for i in range(nb):
                    m = m0 + i
                    nc.tensor.matmul(
                        o_ps[:], vgs[m // 4][:, m % 4, :], p_sb[:, :, i, :],
                        start=(first and i == 0), stop=(m == nb_tot - 1))

            def epilogue(c, o_ps):
                # POOL DGE so result stores don't head-of-line block the SP
                # sequencer issuing input stage loads
                o_sb = pwork.tile([DK + 1, CH], F32, tag="osb", name="o_sb")
                nc.vector.tensor_copy(o_sb[:], o_ps[:])
                eng = nc.sync if c >= NCH - 2 else nc.gpsimd
                eng.dma_start(out_d[c], o_sb[:])

            def chunk_pair(c0, c1):
                """Interleave the score/exp/PV units of two chunks so one
                chunk's PE work hides the other's ACT-exp latency (each chunk
                accumulates in its own PSUM bank)."""
                cs = [c for c in (c0, c1) if c is not None]
                units = {c: [(m0, min(4, nkq[c] - m0))
                             for m0 in range(0, nkq[c], 4)] for c in cs}
                ops = {c: ps_o.tile([DK + 1, CH], F32, tag="o",
                                    name=f"o{c % 2}") for c in cs}
                nu = max(len(units[c]) for c in cs)
                for u in range(nu):
                    for c in cs:
                        if u < len(units[c]):
                            m0, nb = units[c][u]
                            emit_unit(c, m0, nb, ops[c], first=(u == 0))
                for c in cs:
                    epilogue(c, ops[c])

            def q_xbars(g):
                return [(qxg[g][:], 4 * g)]

            def k_xbars(g):
                calls = [(kxg[g][0][:], 4 * g), (kxg[g][1][:], 4 * g + 1)]
                if g == NSK - 1:
                    calls.append((kxf[:], NKB))
                return calls

            # projection prefetch schedule: kT pair tile m+1 (k group
            # (m+1)//4) is needed by chunk m, so k group g feeds chunks
            # >= 4g-1; v group g feeds >= 4g; q group g feeds >= 2g
            pre = {c: [] for c in range(NCH)}
            for g in range(1, NSK):
                pre[max(0, 4 * g - 4)] += [("k", g)]
                pre[max(0, 4 * g - 3)] += [("v", g)]
            for g in range(1, NSQ):  # q groups 1..7 needed at chunk 2g
                pre[max(0, 2 * g - 2)] += [("q", g)]

            project_qk("q", qt3, wq_sb, qnat, 0, 0, q_xbars(0), sts=head_q0)
            project_qk("k", kt3, wk_sb, knat, 1, 0, k_xbars(0), sts=head_k0)
            project_v(0, sts=head_v0)
            for c0 in range(0, NCH, 2):
                c1 = c0 + 1
                # groups the current pair depends on must be fully emitted
                needed = {("q", g) for g in range(c1 // 2 + 1)}
                needed |= {("k", g) for g in range(min(NSK, (c1 + 1) // 4 + 1))}
                needed |= {("v", g) for g in range(c1 // 4 + 1)}
                while bg and bg[0][0] in needed:
                    bg.pop(0)[1]()
                # stage + enqueue projections for upcoming chunks; their
                # matmuls are woven between this pair's attention units
                for c in (c0, c1):
                    for kind, g in pre[c]:
                        if kind == "q":
                            project_qk("q", qt3, wq_sb, qnat, 0, g,
                                       q_xbars(g), defer=True)
                        elif kind == "k":
                            project_qk("k", kt3, wk_sb, knat, 1, g,
                                       k_xbars(g), defer=True)
                        else:
                            project_v(g, defer=True)
                chunk_pair(c0, c1)
            while bg:
                bg.pop(0)[1]()

    nc.compile()
    return nc


def _get_prog(variant):
    if variant not in _prog_cache:
        _prog_cache[variant] = _build(variant)
    return _prog_cache[variant]


def kernel(queries, keys, values, Wq, Wk, Wv, mask):
    import ml_dtypes  # noqa: F401  registers numpy bfloat16/fp8

    bf16 = np.dtype(mybir.dt.np(BF16))
    e3m4 = np.dtype(mybir.dt.np(E3))
    queries = np.asarray(queries, dtype=np.float32)
    keys = np.asarray(keys, dtype=np.float32)
    values = np.asarray(values, dtype=np.float32)
    mask_np = np.asarray(mask)

    causal = bool(np.array_equal(
        mask_np != 0, np.tril(np.ones((S, S), dtype=bool))))
    full = bool((mask_np != 0).all()) if not causal else False
    if not (causal or full):
        raise NotImplementedError("general mask not supported")
    variant = "causal" if causal else "full"

    qt = np.ascontiguousarray(queries.transpose(0, 2, 1)).astype(e3m4)
    kt = np.ascontiguousarray(keys.transpose(0, 2, 1)).astype(e3m4)
    vt = np.ascontiguousarray(values.transpose(0, 2, 1)).astype(bf16)
    wq = np.ascontiguousarray(
        np.asarray(Wq, dtype=np.float32).T * WSCALE).astype(e3m4)
    wk = np.ascontiguousarray(
        np.asarray(Wk, dtype=np.float32).T * WSCALE).astype(e3m4)
    wv = np.ascontiguousarray(np.asarray(Wv, dtype=np.float32).T).astype(bf16)

    in_maps = []
    for core in range(NCORES):
        b, h = divmod(core, 2)
        ksel = np.ascontiguousarray(
            kt[b].reshape(D, S // JB, JB)[:, h::2, :].reshape(D, KB))
        vsel = np.ascontiguousarray(
            vt[b].reshape(D, S // JB, JB)[:, h::2, :].reshape(D, KB))
        m = {"qt": qt[b], "kt": ksel, "vt": vsel,
             "wq": wq, "wk": wk, "wv": wv}
        if variant == "causal":
            i = np.arange(CH)[None, :]
            j = np.arange(JB)[:, None]
            m["maskb"] = ((i - j - JB * h) >= 0).astype(np.float32).astype(bf16).reshape(JB, 2, JB)
        in_maps.append(m)

    nc = _get_prog(variant)
    res = run_bass_kernel_spmd(nc, in_maps, list(range(NCORES)))

    out = np.empty((B, S, DK), dtype=np.float32)
    ov = out.reshape(B, NCH, CH, DK)
    for b in range(B):
        r0 = res.results[2 * b]["out"]  # [NCH, DK+1, CH]
        r1 = res.results[2 * b + 1]["out"]
        tot = r0.astype(np.float64) + r1.astype(np.float64)
        ov[b] = (tot[:, :DK, :] / tot[:, DK:DK + 1, :]).transpose(0, 2, 1)
    return out


if __name__ == "__main__":
    rng = np.random.default_rng(0)
    q = rng.standard_normal((B, S, D), dtype=np.float32)
    k = rng.standard_normal((B, S, D), dtype=np.float32)
    v = rng.standard_normal((B, S, D), dtype=np.float32)
    sc = 1.0 / np.sqrt(D)
    wq = rng.uniform(-sc, sc, (DK, D)).astype(np.float32)
    wk = rng.uniform(-sc, sc, (DK, D)).astype(np.float32)
    wv = rng.uniform(-sc, sc, (DK, D)).astype(np.float32)
    msk = np.tril(np.ones((S, S), dtype=np.int32))
    out = kernel(queries=q, keys=k, values=v, Wq=wq, Wk=wk, Wv=wv, mask=msk)
    print("out", out.shape, out.dtype, float(np.abs(out).mean()))
